# revision 24
# baseline (speedup 1.0000x reference)
"""Trainium2 Bass kernel for nn_Encoder_72026601554062 (6-layer dense transformer
encoder, B=8 T=1024 DM=768 H=12 DK=DV=64 DH=3072).

Sharding: pure data-parallel over batch - 1 sequence per NeuronCore, weights
replicated, no collectives.

v2 redesign vs the original baseline (3.03ms):
- residual stream kept in bf16 (2x DVE rate, no CAST passes, half the SBUF)
- attention softmax denominators inverted with reciprocal_approx_fast per head
  (was: full-precision reciprocal on a [1,T] single-lane shape, 6.5us x 84)
- LN rstd via exp(-0.5*ln(var+eps)) on the scalar engine so the whole kernel
  uses ONE activation table set (exp/ln) - no table switches
- LN squares on the vector engine (tensor_mul) instead of ACT Square
- post-attention phase (out-proj -> LN1 -> FFN -> LN2 -> QKV of next layer)
  pipelined in T-halves so DVE LayerNorm work hides under PE matmuls
- weights for layer l+1 prefetched during attention(l)
- single 7-bank PSUM footprint in the pipelined phase (pf0-2/mm/st tags)
"""

import numpy as np

L, H, DK, DV, DM, DH = 6, 12, 64, 64, 768, 3072
B, T = 8, 1024
N_CORES = 8
KD = DM // 128   # 6
KH = DH // 128   # 24
KT = T // 128    # 8
NT = T // 512    # 2
TH = 512         # T-half size
SCALE = DM ** 0.5
HV = DV + 1      # per-head V width incl. ones column


def _pos_embed():
    pos = np.arange(T, dtype=np.float32)[:, None]
    i = np.arange(DM)[None, :]
    exp = ((i // 2) * 2).astype(np.float32) / DM
    ang = pos / np.power(np.float32(10000.0), exp, dtype=np.float32)
    return np.where(i % 2 == 0, np.sin(ang), np.cos(ang)).astype(np.float32)


def _build(nl=L, debug=False):
    import concourse.tile as tile
    from concourse import bacc, mybir
    from contextlib import ExitStack

    f32 = mybir.dt.float32
    bf16 = mybir.dt.bfloat16
    f8 = mybir.dt.float8e4
    AF = mybir.ActivationFunctionType
    ALU = mybir.AluOpType
    DR = mybir.MatmulPerfMode.DoubleRow
    QKS = 32.0           # fp8 scale for q/k (cancels via exp scale)
    VS = 16.0            # fp8 scale for v and the ones column
    PTS = float(np.log(8.0))  # exp bias: pt = 8*exp(s) (cancels in normalize)
    HVP = 784            # padded KT row width (HV*H=780 -> %16 for DoubleRow)

    nc = bacc.Bacc("TRN2", target_bir_lowering=False, num_devices=N_CORES)

    xt_d = nc.dram_tensor("xt", [DM, T], f32, kind="ExternalInput")
    wq_d = nc.dram_tensor("wq", [nl, DM, H * DK], bf16, kind="ExternalInput")
    wk_d = nc.dram_tensor("wk", [nl, DM, H * DK], bf16, kind="ExternalInput")
    wv_d = nc.dram_tensor("wv", [nl, DM, H * DV], bf16, kind="ExternalInput")
    pw_d = nc.dram_tensor("pw", [nl, H * DV, DM], bf16, kind="ExternalInput")
    w1_d = nc.dram_tensor("w1", [nl, DM, DH], bf16, kind="ExternalInput")
    w2_d = nc.dram_tensor("w2", [nl, DH, DM], bf16, kind="ExternalInput")
    pb_d = nc.dram_tensor("pb", [nl, DM], f32, kind="ExternalInput")
    b1_d = nc.dram_tensor("b1", [nl, DH], f32, kind="ExternalInput")
    b2_d = nc.dram_tensor("b2", [nl, DM], f32, kind="ExternalInput")
    l1g_d = nc.dram_tensor("l1g", [nl, DM], f32, kind="ExternalInput")
    l1b_d = nc.dram_tensor("l1b", [nl, DM], f32, kind="ExternalInput")
    l2g_d = nc.dram_tensor("l2g", [nl, DM], f32, kind="ExternalInput")
    l2b_d = nc.dram_tensor("l2b", [nl, DM], f32, kind="ExternalInput")
    yt_d = nc.dram_tensor("yt", [DM, T], f32, kind="ExternalOutput")
    dbg = {}
    if debug:
        f8_ = mybir.dt.float8e4
        for nm, shape, dt in (("qT", [DM, T], f8_), ("kT", [DM, T], f8_),
                              ("va", [128, KT * 784], f8_),
                              ("oT", [DM, T], bf16), ("r1", [DM, T], bf16),
                              ("xln1", [DM, T], bf16), ("r2", [DM, T], bf16)):
            dbg[nm] = nc.dram_tensor(f"dbg_{nm}", shape, dt, kind="ExternalOutput")

    def vec_ap(d, l):  # [nl, DM] dram row l -> [128, KD]
        return d[l].rearrange("(k p) -> p k", p=128)

    def hs(c):
        return slice(c * TH, (c + 1) * TH)

    with tile.TileContext(nc) as tc, ExitStack() as ctx:
        const = ctx.enter_context(tc.tile_pool(name="const", bufs=1))
        prm = ctx.enter_context(tc.tile_pool(name="prm", bufs=2))
        strm = ctx.enter_context(tc.tile_pool(name="strm", bufs=3))
        qkp = ctx.enter_context(tc.tile_pool(name="qkp", bufs=1))
        vap = ctx.enter_context(tc.tile_pool(name="vap", bufs=1))
        otp = ctx.enter_context(tc.tile_pool(name="otp", bufs=1))
        wbig = ctx.enter_context(tc.tile_pool(name="wbig", bufs=1))
        fwp = ctx.enter_context(tc.tile_pool(name="fwp", bufs=2))
        htp = ctx.enter_context(tc.tile_pool(name="htp", bufs=1))
        sqp = ctx.enter_context(tc.tile_pool(name="sqp", bufs=1))
        stat = ctx.enter_context(tc.tile_pool(name="stat", bufs=1))
        aux = ctx.enter_context(tc.tile_pool(name="aux", bufs=2))
        bcp = ctx.enter_context(tc.tile_pool(name="bcp", bufs=1))
        nrm = ctx.enter_context(tc.tile_pool(name="nrm", bufs=2))
        ppool = ctx.enter_context(tc.tile_pool(name="ppool", bufs=3))
        lnt = ctx.enter_context(tc.tile_pool(name="lnt", bufs=1))
        rfp = ctx.enter_context(tc.tile_pool(name="rfp", bufs=1))

        ones_b = const.tile([128, 1], bf16)
        nc.vector.memset(ones_b, 1.0)
        eps_sb = const.tile([1, 1], f32)
        nc.vector.memset(eps_sb, 1e-5)
        ln8_sb = const.tile([128, 1], f32)
        nc.vector.memset(ln8_sb, PTS)

        def load_layer_params(l):
            lp = prm.tile([128, 7 * KD], f32, tag="lp", name="lp")
            for i, d in enumerate((pb_d, b2_d, l1g_d, l1b_d, l2g_d, l2b_d)):
                nc.sync.dma_start(out=lp[:, i * KD:(i + 1) * KD], in_=vec_ap(d, l))
            nc.vector.tensor_scalar_mul(
                lp[:, 6 * KD:7 * KD], lp[:, 2 * KD:3 * KD], -1.0)
            b1_sb = prm.tile([128, KH + KD], f32, tag="b1", name="b1sb")
            nc.sync.dma_start(
                out=b1_sb[:, 0:KH], in_=b1_d[l].rearrange("(k p) -> p k", p=128))
            nc.vector.tensor_scalar_mul(
                b1_sb[:, KH:KH + KD], lp[:, 4 * KD:5 * KD], -1.0)
            return {
                "pb": lp[:, 0:KD], "b2": lp[:, KD:2 * KD],
                "g1": lp[:, 2 * KD:3 * KD], "bb1": lp[:, 3 * KD:4 * KD],
                "g2": lp[:, 4 * KD:5 * KD], "bb2": lp[:, 5 * KD:6 * KD],
                "gneg1": lp[:, 6 * KD:7 * KD], "gneg2": b1_sb[:, KH:KH + KD],
                "b1": b1_sb[:, 0:KH],
            }

        def load_qkvw(l, names=("wq", "wk", "wv", "pw")):
            dmap = {"wq": wq_d, "wk": wk_d, "wv": wv_d, "pw": pw_d}
            w = {}
            for nm in names:
                t = wbig.tile([128, KD, DM], bf16, tag=nm, name=f"{nm}{l}")
                nc.sync.dma_start(
                    out=t, in_=dmap[nm][l].rearrange("(k p) m -> p k m", p=128))
                w[nm] = t
            return w

        def qkv_half(c, xsrc, qT, kT, va, wq, wk, wv, psB):
            for w_sb, dst in ((wq, qT), (wk, kT)):
                for m in range(KD):
                    ps = psB.tile([128, TH], f32, tag="mm", bufs=2, name="psa")
                    for k in range(KD):
                        nc.tensor.matmul(
                            ps, w_sb[:, k, m * 128:(m + 1) * 128],
                            xsrc[:, k, hs(c)],
                            start=(k == 0), stop=(k == KD - 1))
                    nc.vector.tensor_scalar_mul(dst[:, m, hs(c)], ps, QKS)
            for tk in range(c * 4, c * 4 + 4):
                # v in token-major layout, interleaved into va; two psum chunks
                for n0, nw, h0, hn in ((0, 512, 0, 8), (512, 256, 8, 4)):
                    ps = psB.tile([128, TH], f32, tag="mm", bufs=2, name="psv")
                    for k in range(KD):
                        nc.tensor.matmul(
                            ps[:, 0:nw], xsrc[:, k, tk * 128:(tk + 1) * 128],
                            wv[:, k, n0:n0 + nw],
                            start=(k == 0), stop=(k == KD - 1))
                    out_ap = va[:, tk, 0:780].rearrange(
                        "p (h v) -> p h v", v=HV)[:, h0:h0 + hn, 0:64]
                    in_ap = ps[:, 0:nw].rearrange("p (h v) -> p h v", v=64)
                    nc.vector.tensor_scalar_mul(out_ap, in_ap, VS)

        def attention(qT, kT, va, oT, psS, psO):
            for h in range(H):
                d, off = divmod(h, 2)
                off *= 64
                po = psO.tile([65, T], f32, tag="po", name="po")
                pts = []

                def st_step(tk, d=d, off=off, pts=pts):
                    ps = psS.tile([128, T], f32, tag="pss", name="pss")
                    for n in range(NT):
                        nc.tensor.matmul(
                            ps[:, n * 512:(n + 1) * 512],
                            kT[off:off + 64, d, tk * 128:(tk + 1) * 128],
                            qT[off:off + 64, d, n * 512:(n + 1) * 512])
                    if tk % 2 == 0:
                        pts.append(ppool.tile([128, 2, T], f8, tag="pt",
                                              bufs=2, name="pt"))
                    # scores carry a QKS^2 factor; pt = 8*exp(s/SCALE)
                    nc.scalar.activation(
                        pts[tk // 2][:, tk % 2, :], ps, AF.Exp,
                        scale=1.0 / (QKS * QKS * SCALE), bias=ln8_sb[:])

                def pv_pair(j, h=h, po=po, pts=pts):
                    for n in range(NT):
                        nc.tensor.matmul(
                            po[:, n * 512:(n + 1) * 512],
                            va[:, 2 * j:2 * j + 2, h * HV:(h + 1) * HV],
                            pts[j][:, :, n * 512:(n + 1) * 512],
                            perf_mode=DR,
                            start=(j == 0), stop=(j == KT // 2 - 1))

                st_step(0)
                st_step(1)
                st_step(2)
                st_step(3)
                for j in range(KT // 2 - 1):
                    pv_pair(j)
                    if 2 * j + 4 < KT:
                        st_step(2 * j + 4)
                        st_step(2 * j + 5)
                pv_pair(KT // 2 - 1)
                # custom-DVE ops misread PSUM on HW - stage the denominator
                # row through SBUF before the approx reciprocal
                dn = nrm.tile([1, T], f32, tag="dn", bufs=1, name="dn")
                nc.vector.tensor_copy(dn, po[64:65, :])
                rp = nrm.tile([1, T], f32, tag="rp", bufs=1, name="rp")
                nc.vector.reciprocal_approx_fast(out=rp, in_=dn)
                rb = nrm.tile([64, T], f32, tag="rb", bufs=1, name="rb")
                nc.gpsimd.partition_broadcast(rb, rp)
                nc.vector.tensor_mul(oT[off:off + 64, d, :], po[0:64, :], rb)

        def outproj_half(c, oT, pw, pb_sb, rf, r_b, psB):
            # rf holds the f32 carrier (LN2 output of the previous layer);
            # update it in place: rf = (proj + pb) + rf.  r_b gets the bf16
            # copy the LN stats matmuls read.
            for m in range(KD):
                ps = psB.tile([128, TH], f32, tag="mm", bufs=2, name="psc")
                for k in range(KD):
                    nc.tensor.matmul(
                        ps, pw[:, k, m * 128:(m + 1) * 128], oT[:, k, hs(c)],
                        start=(k == 0), stop=(k == KD - 1))
                nc.vector.scalar_tensor_tensor(
                    out=rf[:, m, hs(c)], in0=ps, scalar=pb_sb[:, m:m + 1],
                    in1=rf[:, m, hs(c)], op0=ALU.add, op1=ALU.add)
                nc.vector.tensor_copy(r_b[:, m, hs(c)], rf[:, m, hs(c)])

        def ln_stats_half(c, r, psB):
            """returns (rs_bc, mu_bc) broadcast tiles for this half."""
            sq = sqp.tile([128, KD, TH], bf16, tag="sq", name="sq")
            nc.vector.tensor_mul(sq, r[:, :, hs(c)], r[:, :, hs(c)])
            s1p = psB.tile([1, TH], f32, tag="st", bufs=2, name="s1p")
            s2p = psB.tile([1, TH], f32, tag="st", bufs=2, name="s2p")
            for k in range(KD):
                nc.tensor.matmul(s1p, ones_b, r[:, k, hs(c)],
                                 start=(k == 0), stop=(k == KD - 1))
            for k in range(KD):
                nc.tensor.matmul(s2p, ones_b, sq[:, k, :],
                                 start=(k == 0), stop=(k == KD - 1))
            mean = stat.tile([1, TH], f32, tag="mean", name="mean")
            nc.vector.tensor_scalar_mul(mean, s1p, 1.0 / DM)
            ms = aux.tile([1, TH], f32, tag="aux", name="ms")
            nc.vector.tensor_scalar_mul(ms, s2p, 1.0 / DM)
            var = aux.tile([1, TH], f32, tag="aux", name="var")
            nc.vector.tensor_mul(var, mean, mean)
            nc.vector.tensor_sub(var, ms, var)
            # rstd = exp(-0.5 * ln(var + eps)) - stays in the exp/ln table set
            lnv = aux.tile([1, TH], f32, tag="aux", name="lnv")
            nc.scalar.activation(lnv, var, AF.Ln, bias=eps_sb[:])
            rstd = aux.tile([1, TH], f32, tag="aux", name="rstd")
            nc.scalar.activation(rstd, lnv, AF.Exp, scale=-0.5)
            murs = stat.tile([1, TH], f32, tag="murs", name="murs")
            nc.vector.tensor_mul(murs, mean, rstd)
            rs_bc = bcp.tile([128, TH], f32, tag="rs_bc", name="rs_bc")
            nc.gpsimd.partition_broadcast(rs_bc, rstd)
            mu_bc = bcp.tile([128, TH], f32, tag="mu_bc", name="mu_bc")
            nc.gpsimd.partition_broadcast(mu_bc, murs)
            return rs_bc, mu_bc

        def ln_apply_half(c, rf, rs_bc, mu_bc, g_sb, gneg_sb, b_sb, out_b,
                          yt_out=False):
            """rf[:,d,half] = ((rf*g)*rstd - g*mu*rstd) + b (in-place carrier
            update, f32); out_b gets the bf16 copy for matmul consumers.
            yt_out: stream the f32 carrier slice to yt dram (final layer)."""
            for d in range(KD):
                t = lnt.tile([128, TH], f32, tag="lt", name="lt")
                nc.vector.scalar_tensor_tensor(
                    out=t, in0=rf[:, d, hs(c)], scalar=g_sb[:, d:d + 1],
                    in1=rs_bc, op0=ALU.mult, op1=ALU.mult)
                u = lnt.tile([128, TH], f32, tag="lu", name="lu")
                nc.vector.scalar_tensor_tensor(
                    out=u, in0=mu_bc, scalar=gneg_sb[:, d:d + 1],
                    in1=t, op0=ALU.mult, op1=ALU.add)
                nc.vector.tensor_scalar(rf[:, d, hs(c)], u,
                                        b_sb[:, d:d + 1], None, ALU.add)
                if yt_out:
                    nc.sync.dma_start(
                        out=yt_d[:].rearrange(
                            "(k p) t -> p k t", p=128)[:, d, hs(c)],
                        in_=rf[:, d, hs(c)])
                else:
                    nc.vector.tensor_copy(out_b[:, d, hs(c)], rf[:, d, hs(c)])

        def ffn1_half(c, xln1, b1_sb, hT, l, psB):
            for j in range(12):  # 12 chunks of 2 m-columns each
                w1t = fwp.tile([128, KD, 256], bf16, tag="w1t", name="w1t")
                nc.sync.dma_start(
                    out=w1t,
                    in_=w1_d[l].rearrange(
                        "(k p) (a m) -> p k a m", p=128, m=256)[:, :, j, :])
                for mm in range(2):
                    m = j * 2 + mm
                    ps = psB.tile([128, TH], f32, tag="mm", bufs=2, name="pse")
                    for k in range(KD):
                        nc.tensor.matmul(
                            ps, w1t[:, k, mm * 128:(mm + 1) * 128],
                            xln1[:, k, hs(c)],
                            start=(k == 0), stop=(k == KD - 1))
                    nc.vector.tensor_scalar(
                        hT[:, m, :], ps, b1_sb[:, m:m + 1], 0.0,
                        ALU.add, ALU.max)

        def ffn2_half(c, hT, b2_sb, rf, r2b, l, psB):
            for g in range(2):  # two m-groups of 3 -> only 3 psum banks
                pf = [psB.tile([128, TH], f32, tag=f"pf{i}", name=f"pf{i}")
                      for i in range(3)]
                for kb in range(8):  # 8 chunks of 3 dh-rows each
                    w2t = fwp.tile([128, 3, 384], bf16, tag="w2t", name="w2t")
                    nc.sync.dma_start(
                        out=w2t,
                        in_=w2_d[l].rearrange(
                            "(b k p) m -> p b k m", k=3, p=128)
                        [:, kb, :, g * 384:(g + 1) * 384])
                    for k in range(3):
                        for i in range(3):
                            nc.tensor.matmul(
                                pf[i], w2t[:, k, i * 128:(i + 1) * 128],
                                hT[:, kb * 3 + k, :],
                                start=(kb == 0 and k == 0),
                                stop=(kb == 7 and k == 2))
                for i in range(3):
                    m = g * 3 + i
                    nc.vector.scalar_tensor_tensor(
                        out=rf[:, m, hs(c)], in0=pf[i],
                        scalar=b2_sb[:, m:m + 1],
                        in1=rf[:, m, hs(c)], op0=ALU.add, op1=ALU.add)
                    nc.vector.tensor_copy(r2b[:, m, hs(c)], rf[:, m, hs(c)])

        # ---- kernel start: load x, convert to bf16, first-layer weights ----
        w = load_qkvw(0)
        prms = load_layer_params(0)
        # rf is the persistent f32 residual carrier; starts as the input
        rf = rfp.tile([128, KD, T], f32, tag="rf", name="rf")
        nc.sync.dma_start(out=rf, in_=xt_d[:].rearrange("(k p) t -> p k t", p=128))
        xb = strm.tile([128, KD, T], bf16, tag="strm", name="xb0")
        nc.scalar.copy(xb, rf)

        qT = qkp.tile([128, KD, T], f8, tag="qT", name="qT")
        kT = qkp.tile([128, KD, T], f8, tag="kT", name="kT")
        va = vap.tile([128, KT, HVP], f8, tag="va", name="va")
        nc.vector.memset(
            va[:, :, 0:780].rearrange("p c (h v) -> p c h v", v=HV)[:, :, :, 64],
            VS)
        with tc.tile_pool(name="psB0", bufs=1, space="PSUM") as psB0:
            for c in range(2):
                qkv_half(c, xb, qT, kT, va, w["wq"], w["wk"], w["wv"], psB0)

        for l in range(nl):
            if debug and l == 0:
                nc.sync.dma_start(
                    out=dbg["qT"][:].rearrange("(k p) t -> p k t", p=128), in_=qT)
                nc.sync.dma_start(
                    out=dbg["kT"][:].rearrange("(k p) t -> p k t", p=128), in_=kT)
                nc.sync.dma_start(
                    out=dbg["va"][:].rearrange("p (c m) -> p c m", m=HVP),
                    in_=va)
            # ---- attention ----
            oT = otp.tile([128, KD, T], bf16, tag="oT", name="oT")
            pw_cur = w["pw"]
            with tc.tile_pool(name="psS", bufs=2, space="PSUM") as psS, \
                 tc.tile_pool(name="psO", bufs=2, space="PSUM") as psO:
                attention(qT, kT, va, oT, psS, psO)
            if debug and l == 0:
                nc.sync.dma_start(
                    out=dbg["oT"][:].rearrange("(k p) t -> p k t", p=128), in_=oT)
            # prefetch next layer weights + params (lands during attention/B).
            # pw(l+1) must wait: its buffer (bufs=1) is still read by this
            # layer's out-proj below - prefetch it after outproj_half(1).
            if l + 1 < nl:
                w = load_qkvw(l + 1, names=("wq", "wk", "wv"))
                next_prms = load_layer_params(l + 1)

            # ---- post-attention, pipelined by T-halves ----
            r1b = strm.tile([128, KD, T], bf16, tag="strm", name=f"r1_{l}")
            xln1 = strm.tile([128, KD, T], bf16, tag="strm", name=f"xln1_{l}")
            r2b = strm.tile([128, KD, T], bf16, tag="strm", name=f"r2_{l}")
            last = l + 1 == nl
            if not last:
                xn = strm.tile([128, KD, T], bf16, tag="strm", name=f"x_{l + 1}")
                qT = qkp.tile([128, KD, T], f8, tag="qT", name=f"qT{l + 1}")
                kT = qkp.tile([128, KD, T], f8, tag="kT", name=f"kT{l + 1}")
                va = vap.tile([128, KT, HVP], f8, tag="va", name=f"va{l + 1}")
                nc.vector.memset(
                    va[:, :, 0:780].rearrange(
                        "p c (h v) -> p c h v", v=HV)[:, :, :, 64], VS)
            else:
                xn = None

            with tc.tile_pool(name="psB", bufs=1, space="PSUM") as psB:
                outproj_half(0, oT, pw_cur, prms["pb"], rf, r1b, psB)
                st1_0 = ln_stats_half(0, r1b, psB)
                outproj_half(1, oT, pw_cur, prms["pb"], rf, r1b, psB)
                if l + 1 < nl:
                    w.update(load_qkvw(l + 1, names=("pw",)))
                ln_apply_half(0, rf, *st1_0, prms["g1"], prms["gneg1"],
                              prms["bb1"], xln1)
                st1_1 = ln_stats_half(1, r1b, psB)
                hT = htp.tile([128, KH, TH], bf16, tag="hT", name="hT0")
                ffn1_half(0, xln1, prms["b1"], hT, l, psB)
                ln_apply_half(1, rf, *st1_1, prms["g1"], prms["gneg1"],
                              prms["bb1"], xln1)
                ffn2_half(0, hT, prms["b2"], rf, r2b, l, psB)
                hT = htp.tile([128, KH, TH], bf16, tag="hT", name="hT1")
                ffn1_half(1, xln1, prms["b1"], hT, l, psB)
                st2_0 = ln_stats_half(0, r2b, psB)
                ln_apply_half(0, rf, *st2_0, prms["g2"], prms["gneg2"],
                              prms["bb2"], xn, yt_out=last)
                if not last:
                    qkv_half(0, xn, qT, kT, va, w["wq"], w["wk"], w["wv"], psB)
                ffn2_half(1, hT, prms["b2"], rf, r2b, l, psB)
                st2_1 = ln_stats_half(1, r2b, psB)
                ln_apply_half(1, rf, *st2_1, prms["g2"], prms["gneg2"],
                              prms["bb2"], xn, yt_out=last)
                if not last:
                    qkv_half(1, xn, qT, kT, va, w["wq"], w["wk"], w["wv"], psB)
                if debug and l == 0:
                    for nm, t in (("r1", r1b), ("xln1", xln1), ("r2", r2b)):
                        nc.sync.dma_start(
                            out=dbg[nm][:].rearrange("(k p) t -> p k t", p=128),
                            in_=t)
            if not last:
                prms = next_prms

    nc.compile()
    return nc


_NC = None


def _get_nc():
    global _NC
    if _NC is None:
        _NC = _build()
    return _NC


def _prep_inputs(inputs, nl=L):
    import ml_dtypes
    bf = ml_dtypes.bfloat16
    gi = lambda k: np.asarray(inputs[k])
    x = gi("x").astype(np.float32)
    wq, wk, wv = gi("wq"), gi("wk"), gi("wv")
    pe = _pos_embed()
    shared = {
        "wq": np.ascontiguousarray(wq[:nl].transpose(0, 2, 1, 3).reshape(nl, DM, H * DK)).astype(bf),
        "wk": np.ascontiguousarray(wk[:nl].transpose(0, 2, 1, 3).reshape(nl, DM, H * DK)).astype(bf),
        "wv": np.ascontiguousarray(wv[:nl].transpose(0, 2, 1, 3).reshape(nl, DM, H * DV)).astype(bf),
        "pw": np.ascontiguousarray(gi("proj_w")[:nl]).astype(bf),
        "w1": np.ascontiguousarray(gi("w1")[:nl]).astype(bf),
        "w2": np.ascontiguousarray(gi("w2")[:nl]).astype(bf),
        "pb": np.ascontiguousarray(gi("proj_b")[:nl], dtype=np.float32),
        "b1": np.ascontiguousarray(gi("b1")[:nl], dtype=np.float32),
        "b2": np.ascontiguousarray(gi("b2")[:nl], dtype=np.float32),
        "l1g": np.ascontiguousarray(gi("ln1_g")[:nl], dtype=np.float32),
        "l1b": np.ascontiguousarray(gi("ln1_b")[:nl], dtype=np.float32),
        "l2g": np.ascontiguousarray(gi("ln2_g")[:nl], dtype=np.float32),
        "l2b": np.ascontiguousarray(gi("ln2_b")[:nl], dtype=np.float32),
    }
    in_maps = []
    for b in range(B):
        m = dict(shared)
        m["xt"] = np.ascontiguousarray((x[b] + pe).T.astype(np.float32))
        in_maps.append(m)
    return in_maps


def run(inputs, trace=False):
    from concourse.bass_utils import run_bass_kernel_spmd
    nc = _get_nc()
    in_maps = _prep_inputs(inputs)
    res = run_bass_kernel_spmd(nc, in_maps, list(range(N_CORES)), trace=trace)
    out = np.stack([res.results[b]["yt"].T for b in range(B)]).astype(np.float32)
    return out, res


def kernel(**inputs):
    out, _ = run(inputs)
    return out


# revision 25
# speedup vs baseline: 1.0008x; 1.0008x over previous
"""Trainium2 Bass kernel for nn_Encoder_72026601554062 (6-layer dense transformer
encoder, B=8 T=1024 DM=768 H=12 DK=DV=64 DH=3072).

Sharding: pure data-parallel over batch - 1 sequence per NeuronCore, weights
replicated, no collectives.

v2 redesign vs the original baseline (3.03ms):
- residual stream kept in bf16 (2x DVE rate, no CAST passes, half the SBUF)
- attention softmax denominators inverted with reciprocal_approx_fast per head
  (was: full-precision reciprocal on a [1,T] single-lane shape, 6.5us x 84)
- LN rstd via exp(-0.5*ln(var+eps)) on the scalar engine so the whole kernel
  uses ONE activation table set (exp/ln) - no table switches
- LN squares on the vector engine (tensor_mul) instead of ACT Square
- post-attention phase (out-proj -> LN1 -> FFN -> LN2 -> QKV of next layer)
  pipelined in T-halves so DVE LayerNorm work hides under PE matmuls
- weights for layer l+1 prefetched during attention(l)
- single 7-bank PSUM footprint in the pipelined phase (pf0-2/mm/st tags)
"""

import numpy as np

L, H, DK, DV, DM, DH = 6, 12, 64, 64, 768, 3072
B, T = 8, 1024
N_CORES = 8
KD = DM // 128   # 6
KH = DH // 128   # 24
KT = T // 128    # 8
NT = T // 512    # 2
TH = 512         # T-half size
SCALE = DM ** 0.5
HV = DV + 1      # per-head V width incl. ones column


def _pos_embed():
    pos = np.arange(T, dtype=np.float32)[:, None]
    i = np.arange(DM)[None, :]
    exp = ((i // 2) * 2).astype(np.float32) / DM
    ang = pos / np.power(np.float32(10000.0), exp, dtype=np.float32)
    return np.where(i % 2 == 0, np.sin(ang), np.cos(ang)).astype(np.float32)


def _build(nl=L, debug=False):
    import concourse.tile as tile
    from concourse import bacc, mybir
    from contextlib import ExitStack

    f32 = mybir.dt.float32
    bf16 = mybir.dt.bfloat16
    f8 = mybir.dt.float8e4
    AF = mybir.ActivationFunctionType
    ALU = mybir.AluOpType
    DR = mybir.MatmulPerfMode.DoubleRow
    QKS = 32.0           # fp8 scale for q/k (cancels via exp scale)
    VS = 16.0            # fp8 scale for v and the ones column
    PTS = float(np.log(8.0))  # exp bias: pt = 8*exp(s) (cancels in normalize)
    HVP = 784            # padded KT row width (HV*H=780 -> %16 for DoubleRow)

    nc = bacc.Bacc("TRN2", target_bir_lowering=False, num_devices=N_CORES)

    xt_d = nc.dram_tensor("xt", [DM, T], f32, kind="ExternalInput")
    wq_d = nc.dram_tensor("wq", [nl, DM, H * DK], bf16, kind="ExternalInput")
    wk_d = nc.dram_tensor("wk", [nl, DM, H * DK], bf16, kind="ExternalInput")
    wv_d = nc.dram_tensor("wv", [nl, DM, H * DV], bf16, kind="ExternalInput")
    pw_d = nc.dram_tensor("pw", [nl, H * DV, DM], bf16, kind="ExternalInput")
    w1_d = nc.dram_tensor("w1", [nl, DM, DH], bf16, kind="ExternalInput")
    w2_d = nc.dram_tensor("w2", [nl, DH, DM], bf16, kind="ExternalInput")
    pb_d = nc.dram_tensor("pb", [nl, DM], f32, kind="ExternalInput")
    b1_d = nc.dram_tensor("b1", [nl, DH], f32, kind="ExternalInput")
    b2_d = nc.dram_tensor("b2", [nl, DM], f32, kind="ExternalInput")
    l1g_d = nc.dram_tensor("l1g", [nl, DM], f32, kind="ExternalInput")
    l1b_d = nc.dram_tensor("l1b", [nl, DM], f32, kind="ExternalInput")
    l2g_d = nc.dram_tensor("l2g", [nl, DM], f32, kind="ExternalInput")
    l2b_d = nc.dram_tensor("l2b", [nl, DM], f32, kind="ExternalInput")
    yt_d = nc.dram_tensor("yt", [DM, T], f32, kind="ExternalOutput")
    dbg = {}
    if debug:
        f8_ = mybir.dt.float8e4
        for nm, shape, dt in (("qT", [DM, T], f8_), ("kT", [DM, T], f8_),
                              ("va", [128, KT * 784], f8_),
                              ("oT", [DM, T], bf16), ("r1", [DM, T], bf16),
                              ("xln1", [DM, T], bf16), ("r2", [DM, T], bf16)):
            dbg[nm] = nc.dram_tensor(f"dbg_{nm}", shape, dt, kind="ExternalOutput")

    def vec_ap(d, l):  # [nl, DM] dram row l -> [128, KD]
        return d[l].rearrange("(k p) -> p k", p=128)

    def hs(c):
        return slice(c * TH, (c + 1) * TH)

    with tile.TileContext(nc) as tc, ExitStack() as ctx:
        const = ctx.enter_context(tc.tile_pool(name="const", bufs=1))
        prm = ctx.enter_context(tc.tile_pool(name="prm", bufs=2))
        strm = ctx.enter_context(tc.tile_pool(name="strm", bufs=3))
        qkp = ctx.enter_context(tc.tile_pool(name="qkp", bufs=1))
        vap = ctx.enter_context(tc.tile_pool(name="vap", bufs=1))
        otp = ctx.enter_context(tc.tile_pool(name="otp", bufs=1))
        wbig = ctx.enter_context(tc.tile_pool(name="wbig", bufs=1))
        fwp = ctx.enter_context(tc.tile_pool(name="fwp", bufs=2))
        htp = ctx.enter_context(tc.tile_pool(name="htp", bufs=1))
        sqp = ctx.enter_context(tc.tile_pool(name="sqp", bufs=1))
        stat = ctx.enter_context(tc.tile_pool(name="stat", bufs=1))
        aux = ctx.enter_context(tc.tile_pool(name="aux", bufs=2))
        bcp = ctx.enter_context(tc.tile_pool(name="bcp", bufs=1))
        nrm = ctx.enter_context(tc.tile_pool(name="nrm", bufs=2))
        ppool = ctx.enter_context(tc.tile_pool(name="ppool", bufs=3))
        lnt = ctx.enter_context(tc.tile_pool(name="lnt", bufs=1))
        rfp = ctx.enter_context(tc.tile_pool(name="rfp", bufs=1))

        ones_b = const.tile([128, 1], bf16)
        nc.vector.memset(ones_b, 1.0)
        eps_sb = const.tile([1, 1], f32)
        nc.vector.memset(eps_sb, 1e-5)
        ln8_sb = const.tile([128, 1], f32)
        nc.vector.memset(ln8_sb, PTS)

        def load_layer_params(l):
            lp = prm.tile([128, 7 * KD], f32, tag="lp", name="lp")
            for i, d in enumerate((pb_d, b2_d, l1g_d, l1b_d, l2g_d, l2b_d)):
                nc.sync.dma_start(out=lp[:, i * KD:(i + 1) * KD], in_=vec_ap(d, l))
            nc.vector.tensor_scalar_mul(
                lp[:, 6 * KD:7 * KD], lp[:, 2 * KD:3 * KD], -1.0)
            b1_sb = prm.tile([128, KH + KD], f32, tag="b1", name="b1sb")
            nc.sync.dma_start(
                out=b1_sb[:, 0:KH], in_=b1_d[l].rearrange("(k p) -> p k", p=128))
            nc.vector.tensor_scalar_mul(
                b1_sb[:, KH:KH + KD], lp[:, 4 * KD:5 * KD], -1.0)
            return {
                "pb": lp[:, 0:KD], "b2": lp[:, KD:2 * KD],
                "g1": lp[:, 2 * KD:3 * KD], "bb1": lp[:, 3 * KD:4 * KD],
                "g2": lp[:, 4 * KD:5 * KD], "bb2": lp[:, 5 * KD:6 * KD],
                "gneg1": lp[:, 6 * KD:7 * KD], "gneg2": b1_sb[:, KH:KH + KD],
                "b1": b1_sb[:, 0:KH],
            }

        def load_qkvw(l, names=("wq", "wk", "wv", "pw")):
            dmap = {"wq": wq_d, "wk": wk_d, "wv": wv_d, "pw": pw_d}
            w = {}
            for nm in names:
                t = wbig.tile([128, KD, DM], bf16, tag=nm, name=f"{nm}{l}")
                nc.sync.dma_start(
                    out=t, in_=dmap[nm][l].rearrange("(k p) m -> p k m", p=128))
                w[nm] = t
            return w

        def qkv_half(c, xsrc, qT, kT, va, wq, wk, wv, psB):
            for w_sb, dst in ((wq, qT), (wk, kT)):
                for m in range(KD):
                    ps = psB.tile([128, TH], f32, tag="mm", bufs=2, name="psa")
                    for k in range(KD):
                        nc.tensor.matmul(
                            ps, w_sb[:, k, m * 128:(m + 1) * 128],
                            xsrc[:, k, hs(c)],
                            start=(k == 0), stop=(k == KD - 1))
                    nc.vector.tensor_scalar_mul(dst[:, m, hs(c)], ps, QKS)
            for tk in range(c * 4, c * 4 + 4):
                # v in token-major layout, interleaved into va; two psum chunks
                for n0, nw, h0, hn in ((0, 512, 0, 8), (512, 256, 8, 4)):
                    ps = psB.tile([128, TH], f32, tag="mm", bufs=2, name="psv")
                    for k in range(KD):
                        nc.tensor.matmul(
                            ps[:, 0:nw], xsrc[:, k, tk * 128:(tk + 1) * 128],
                            wv[:, k, n0:n0 + nw],
                            start=(k == 0), stop=(k == KD - 1))
                    out_ap = va[:, tk, 0:780].rearrange(
                        "p (h v) -> p h v", v=HV)[:, h0:h0 + hn, 0:64]
                    in_ap = ps[:, 0:nw].rearrange("p (h v) -> p h v", v=64)
                    nc.vector.tensor_scalar_mul(out_ap, in_ap, VS)

        def attention(qT, kT, va, oT, psS, psO):
            for h in range(H):
                d, off = divmod(h, 2)
                off *= 64
                po = psO.tile([65, T], f32, tag="po", name="po")
                pts = []

                def st_step(tk, d=d, off=off, pts=pts):
                    ps = psS.tile([128, T], f32, tag="pss", name="pss")
                    for n in range(NT):
                        nc.tensor.matmul(
                            ps[:, n * 512:(n + 1) * 512],
                            kT[off:off + 64, d, tk * 128:(tk + 1) * 128],
                            qT[off:off + 64, d, n * 512:(n + 1) * 512])
                    if tk % 2 == 0:
                        pts.append(ppool.tile([128, 2, T], f8, tag="pt",
                                              bufs=2, name="pt"))
                    # scores carry a QKS^2 factor; pt = 8*exp(s/SCALE)
                    nc.scalar.activation(
                        pts[tk // 2][:, tk % 2, :], ps, AF.Exp,
                        scale=1.0 / (QKS * QKS * SCALE), bias=ln8_sb[:])

                def pv_pair(j, h=h, po=po, pts=pts):
                    for n in range(NT):
                        nc.tensor.matmul(
                            po[:, n * 512:(n + 1) * 512],
                            va[:, 2 * j:2 * j + 2, h * HV:(h + 1) * HV],
                            pts[j][:, :, n * 512:(n + 1) * 512],
                            perf_mode=DR,
                            start=(j == 0), stop=(j == KT // 2 - 1))

                st_step(0)
                st_step(1)
                st_step(2)
                st_step(3)
                for j in range(KT // 2 - 1):
                    pv_pair(j)
                    if 2 * j + 4 < KT:
                        st_step(2 * j + 4)
                        st_step(2 * j + 5)
                pv_pair(KT // 2 - 1)
                # custom-DVE ops misread PSUM on HW - stage the denominator
                # row through SBUF before the approx reciprocal
                dn = nrm.tile([1, T], f32, tag="dn", bufs=1, name="dn")
                nc.vector.tensor_copy(dn, po[64:65, :])
                rp = nrm.tile([1, T], f32, tag="rp", bufs=1, name="rp")
                nc.vector.reciprocal_approx_fast(out=rp, in_=dn)
                rb = nrm.tile([64, T], f32, tag="rb", bufs=1, name="rb")
                nc.gpsimd.partition_broadcast(rb, rp)
                nc.vector.tensor_mul(oT[off:off + 64, d, :], po[0:64, :], rb)

        def outproj_half(c, oT, pw, pb_sb, rf, r_b, psB):
            # rf holds the f32 carrier (LN2 output of the previous layer);
            # update it in place: rf = (proj + pb) + rf.  r_b gets the bf16
            # copy the LN stats matmuls read.
            for m in range(KD):
                ps = psB.tile([128, TH], f32, tag="mm", bufs=2, name="psc")
                for k in range(KD):
                    nc.tensor.matmul(
                        ps, pw[:, k, m * 128:(m + 1) * 128], oT[:, k, hs(c)],
                        start=(k == 0), stop=(k == KD - 1))
                nc.vector.scalar_tensor_tensor(
                    out=rf[:, m, hs(c)], in0=ps, scalar=pb_sb[:, m:m + 1],
                    in1=rf[:, m, hs(c)], op0=ALU.add, op1=ALU.add)
                nc.vector.tensor_copy(r_b[:, m, hs(c)], rf[:, m, hs(c)])

        def ln_stats_half(c, r, psB):
            """returns (rs_bc, mu_bc) broadcast tiles for this half."""
            sq = sqp.tile([128, KD, TH], bf16, tag="sq", name="sq")
            nc.vector.tensor_mul(sq, r[:, :, hs(c)], r[:, :, hs(c)])
            s1p = psB.tile([1, TH], f32, tag="st", bufs=2, name="s1p")
            s2p = psB.tile([1, TH], f32, tag="st", bufs=2, name="s2p")
            for k in range(KD):
                nc.tensor.matmul(s1p, ones_b, r[:, k, hs(c)],
                                 start=(k == 0), stop=(k == KD - 1))
            for k in range(KD):
                nc.tensor.matmul(s2p, ones_b, sq[:, k, :],
                                 start=(k == 0), stop=(k == KD - 1))
            mean = stat.tile([1, TH], f32, tag="mean", name="mean")
            nc.vector.tensor_scalar_mul(mean, s1p, 1.0 / DM)
            ms = aux.tile([1, TH], f32, tag="aux", name="ms")
            nc.vector.tensor_scalar_mul(ms, s2p, 1.0 / DM)
            var = aux.tile([1, TH], f32, tag="aux", name="var")
            nc.vector.tensor_mul(var, mean, mean)
            nc.vector.tensor_sub(var, ms, var)
            # rstd = 1/sqrt(var+eps); Sqrt keeps the ACT table churn to one
            # set-switch per layer (vs Ln+Exp ping-pong), approx recip is cheap
            sd = aux.tile([1, TH], f32, tag="aux", name="sd")
            nc.scalar.activation(sd, var, AF.Sqrt, bias=eps_sb[:])
            rstd = aux.tile([1, TH], f32, tag="aux", name="rstd")
            nc.vector.reciprocal_approx_fast(out=rstd, in_=sd)
            murs = stat.tile([1, TH], f32, tag="murs", name="murs")
            nc.vector.tensor_mul(murs, mean, rstd)
            rs_bc = bcp.tile([128, TH], f32, tag="rs_bc", name="rs_bc")
            nc.gpsimd.partition_broadcast(rs_bc, rstd)
            mu_bc = bcp.tile([128, TH], f32, tag="mu_bc", name="mu_bc")
            nc.gpsimd.partition_broadcast(mu_bc, murs)
            return rs_bc, mu_bc

        def ln_apply_half(c, rf, rs_bc, mu_bc, g_sb, gneg_sb, b_sb, out_b,
                          yt_out=False):
            """rf[:,d,half] = ((rf*g)*rstd - g*mu*rstd) + b (in-place carrier
            update, f32); out_b gets the bf16 copy for matmul consumers.
            yt_out: stream the f32 carrier slice to yt dram (final layer)."""
            for d in range(KD):
                t = lnt.tile([128, TH], f32, tag="lt", name="lt")
                nc.vector.scalar_tensor_tensor(
                    out=t, in0=rf[:, d, hs(c)], scalar=g_sb[:, d:d + 1],
                    in1=rs_bc, op0=ALU.mult, op1=ALU.mult)
                u = lnt.tile([128, TH], f32, tag="lu", name="lu")
                nc.vector.scalar_tensor_tensor(
                    out=u, in0=mu_bc, scalar=gneg_sb[:, d:d + 1],
                    in1=t, op0=ALU.mult, op1=ALU.add)
                nc.vector.tensor_scalar(rf[:, d, hs(c)], u,
                                        b_sb[:, d:d + 1], None, ALU.add)
                if yt_out:
                    nc.sync.dma_start(
                        out=yt_d[:].rearrange(
                            "(k p) t -> p k t", p=128)[:, d, hs(c)],
                        in_=rf[:, d, hs(c)])
                else:
                    nc.vector.tensor_copy(out_b[:, d, hs(c)], rf[:, d, hs(c)])

        def ffn1_half(c, xln1, b1_sb, hT, l, psB):
            for j in range(12):  # 12 chunks of 2 m-columns each
                w1t = fwp.tile([128, KD, 256], bf16, tag="w1t", name="w1t")
                nc.sync.dma_start(
                    out=w1t,
                    in_=w1_d[l].rearrange(
                        "(k p) (a m) -> p k a m", p=128, m=256)[:, :, j, :])
                for mm in range(2):
                    m = j * 2 + mm
                    ps = psB.tile([128, TH], f32, tag="mm", bufs=2, name="pse")
                    for k in range(KD):
                        nc.tensor.matmul(
                            ps, w1t[:, k, mm * 128:(mm + 1) * 128],
                            xln1[:, k, hs(c)],
                            start=(k == 0), stop=(k == KD - 1))
                    nc.vector.tensor_scalar(
                        hT[:, m, :], ps, b1_sb[:, m:m + 1], 0.0,
                        ALU.add, ALU.max)

        def ffn2_half(c, hT, b2_sb, rf, r2b, l, psB):
            for g in range(2):  # two m-groups of 3 -> only 3 psum banks
                pf = [psB.tile([128, TH], f32, tag=f"pf{i}", name=f"pf{i}")
                      for i in range(3)]
                for kb in range(8):  # 8 chunks of 3 dh-rows each
                    w2t = fwp.tile([128, 3, 384], bf16, tag="w2t", name="w2t")
                    nc.sync.dma_start(
                        out=w2t,
                        in_=w2_d[l].rearrange(
                            "(b k p) m -> p b k m", k=3, p=128)
                        [:, kb, :, g * 384:(g + 1) * 384])
                    for k in range(3):
                        for i in range(3):
                            nc.tensor.matmul(
                                pf[i], w2t[:, k, i * 128:(i + 1) * 128],
                                hT[:, kb * 3 + k, :],
                                start=(kb == 0 and k == 0),
                                stop=(kb == 7 and k == 2))
                for i in range(3):
                    m = g * 3 + i
                    nc.vector.scalar_tensor_tensor(
                        out=rf[:, m, hs(c)], in0=pf[i],
                        scalar=b2_sb[:, m:m + 1],
                        in1=rf[:, m, hs(c)], op0=ALU.add, op1=ALU.add)
                    nc.vector.tensor_copy(r2b[:, m, hs(c)], rf[:, m, hs(c)])

        # ---- kernel start: load x, convert to bf16, first-layer weights ----
        w = load_qkvw(0)
        prms = load_layer_params(0)
        # rf is the persistent f32 residual carrier; starts as the input
        rf = rfp.tile([128, KD, T], f32, tag="rf", name="rf")
        nc.sync.dma_start(out=rf, in_=xt_d[:].rearrange("(k p) t -> p k t", p=128))
        xb = strm.tile([128, KD, T], bf16, tag="strm", name="xb0")
        nc.scalar.copy(xb, rf)

        qT = qkp.tile([128, KD, T], f8, tag="qT", name="qT")
        kT = qkp.tile([128, KD, T], f8, tag="kT", name="kT")
        va = vap.tile([128, KT, HVP], f8, tag="va", name="va")
        nc.vector.memset(
            va[:, :, 0:780].rearrange("p c (h v) -> p c h v", v=HV)[:, :, :, 64],
            VS)
        with tc.tile_pool(name="psB0", bufs=1, space="PSUM") as psB0:
            for c in range(2):
                qkv_half(c, xb, qT, kT, va, w["wq"], w["wk"], w["wv"], psB0)

        for l in range(nl):
            if debug and l == 0:
                nc.sync.dma_start(
                    out=dbg["qT"][:].rearrange("(k p) t -> p k t", p=128), in_=qT)
                nc.sync.dma_start(
                    out=dbg["kT"][:].rearrange("(k p) t -> p k t", p=128), in_=kT)
                nc.sync.dma_start(
                    out=dbg["va"][:].rearrange("p (c m) -> p c m", m=HVP),
                    in_=va)
            # ---- attention ----
            oT = otp.tile([128, KD, T], bf16, tag="oT", name="oT")
            pw_cur = w["pw"]
            with tc.tile_pool(name="psS", bufs=2, space="PSUM") as psS, \
                 tc.tile_pool(name="psO", bufs=2, space="PSUM") as psO:
                attention(qT, kT, va, oT, psS, psO)
            if debug and l == 0:
                nc.sync.dma_start(
                    out=dbg["oT"][:].rearrange("(k p) t -> p k t", p=128), in_=oT)
            # prefetch next layer weights + params (lands during attention/B).
            # pw(l+1) must wait: its buffer (bufs=1) is still read by this
            # layer's out-proj below - prefetch it after outproj_half(1).
            if l + 1 < nl:
                w = load_qkvw(l + 1, names=("wq", "wk", "wv"))
                next_prms = load_layer_params(l + 1)

            # ---- post-attention, pipelined by T-halves ----
            r1b = strm.tile([128, KD, T], bf16, tag="strm", name=f"r1_{l}")
            xln1 = strm.tile([128, KD, T], bf16, tag="strm", name=f"xln1_{l}")
            r2b = strm.tile([128, KD, T], bf16, tag="strm", name=f"r2_{l}")
            last = l + 1 == nl
            if not last:
                xn = strm.tile([128, KD, T], bf16, tag="strm", name=f"x_{l + 1}")
                qT = qkp.tile([128, KD, T], f8, tag="qT", name=f"qT{l + 1}")
                kT = qkp.tile([128, KD, T], f8, tag="kT", name=f"kT{l + 1}")
                va = vap.tile([128, KT, HVP], f8, tag="va", name=f"va{l + 1}")
                nc.vector.memset(
                    va[:, :, 0:780].rearrange(
                        "p c (h v) -> p c h v", v=HV)[:, :, :, 64], VS)
            else:
                xn = None

            with tc.tile_pool(name="psB", bufs=1, space="PSUM") as psB:
                outproj_half(0, oT, pw_cur, prms["pb"], rf, r1b, psB)
                st1_0 = ln_stats_half(0, r1b, psB)
                outproj_half(1, oT, pw_cur, prms["pb"], rf, r1b, psB)
                if l + 1 < nl:
                    w.update(load_qkvw(l + 1, names=("pw",)))
                ln_apply_half(0, rf, *st1_0, prms["g1"], prms["gneg1"],
                              prms["bb1"], xln1)
                st1_1 = ln_stats_half(1, r1b, psB)
                hT = htp.tile([128, KH, TH], bf16, tag="hT", name="hT0")
                ffn1_half(0, xln1, prms["b1"], hT, l, psB)
                ln_apply_half(1, rf, *st1_1, prms["g1"], prms["gneg1"],
                              prms["bb1"], xln1)
                ffn2_half(0, hT, prms["b2"], rf, r2b, l, psB)
                hT = htp.tile([128, KH, TH], bf16, tag="hT", name="hT1")
                ffn1_half(1, xln1, prms["b1"], hT, l, psB)
                st2_0 = ln_stats_half(0, r2b, psB)
                ln_apply_half(0, rf, *st2_0, prms["g2"], prms["gneg2"],
                              prms["bb2"], xn, yt_out=last)
                if not last:
                    qkv_half(0, xn, qT, kT, va, w["wq"], w["wk"], w["wv"], psB)
                ffn2_half(1, hT, prms["b2"], rf, r2b, l, psB)
                st2_1 = ln_stats_half(1, r2b, psB)
                ln_apply_half(1, rf, *st2_1, prms["g2"], prms["gneg2"],
                              prms["bb2"], xn, yt_out=last)
                if not last:
                    qkv_half(1, xn, qT, kT, va, w["wq"], w["wk"], w["wv"], psB)
                if debug and l == 0:
                    for nm, t in (("r1", r1b), ("xln1", xln1), ("r2", r2b)):
                        nc.sync.dma_start(
                            out=dbg[nm][:].rearrange("(k p) t -> p k t", p=128),
                            in_=t)
            if not last:
                prms = next_prms

    nc.compile()
    return nc


_NC = None


def _get_nc():
    global _NC
    if _NC is None:
        _NC = _build()
    return _NC


def _prep_inputs(inputs, nl=L):
    import ml_dtypes
    bf = ml_dtypes.bfloat16
    gi = lambda k: np.asarray(inputs[k])
    x = gi("x").astype(np.float32)
    wq, wk, wv = gi("wq"), gi("wk"), gi("wv")
    pe = _pos_embed()
    shared = {
        "wq": np.ascontiguousarray(wq[:nl].transpose(0, 2, 1, 3).reshape(nl, DM, H * DK)).astype(bf),
        "wk": np.ascontiguousarray(wk[:nl].transpose(0, 2, 1, 3).reshape(nl, DM, H * DK)).astype(bf),
        "wv": np.ascontiguousarray(wv[:nl].transpose(0, 2, 1, 3).reshape(nl, DM, H * DV)).astype(bf),
        "pw": np.ascontiguousarray(gi("proj_w")[:nl]).astype(bf),
        "w1": np.ascontiguousarray(gi("w1")[:nl]).astype(bf),
        "w2": np.ascontiguousarray(gi("w2")[:nl]).astype(bf),
        "pb": np.ascontiguousarray(gi("proj_b")[:nl], dtype=np.float32),
        "b1": np.ascontiguousarray(gi("b1")[:nl], dtype=np.float32),
        "b2": np.ascontiguousarray(gi("b2")[:nl], dtype=np.float32),
        "l1g": np.ascontiguousarray(gi("ln1_g")[:nl], dtype=np.float32),
        "l1b": np.ascontiguousarray(gi("ln1_b")[:nl], dtype=np.float32),
        "l2g": np.ascontiguousarray(gi("ln2_g")[:nl], dtype=np.float32),
        "l2b": np.ascontiguousarray(gi("ln2_b")[:nl], dtype=np.float32),
    }
    in_maps = []
    for b in range(B):
        m = dict(shared)
        m["xt"] = np.ascontiguousarray((x[b] + pe).T.astype(np.float32))
        in_maps.append(m)
    return in_maps


def run(inputs, trace=False):
    from concourse.bass_utils import run_bass_kernel_spmd
    nc = _get_nc()
    in_maps = _prep_inputs(inputs)
    res = run_bass_kernel_spmd(nc, in_maps, list(range(N_CORES)), trace=trace)
    out = np.stack([res.results[b]["yt"].T for b in range(B)]).astype(np.float32)
    return out, res


def kernel(**inputs):
    out, _ = run(inputs)
    return out


# revision 28
# speedup vs baseline: 1.0368x; 1.0361x over previous
"""Trainium2 Bass kernel for nn_Encoder_72026601554062 (6-layer dense transformer
encoder, B=8 T=1024 DM=768 H=12 DK=DV=64 DH=3072).

Sharding: pure data-parallel over batch - 1 sequence per NeuronCore, weights
replicated, no collectives.

v2 redesign vs the original baseline (3.03ms):
- residual stream kept in bf16 (2x DVE rate, no CAST passes, half the SBUF)
- attention softmax denominators inverted with reciprocal_approx_fast per head
  (was: full-precision reciprocal on a [1,T] single-lane shape, 6.5us x 84)
- LN rstd via exp(-0.5*ln(var+eps)) on the scalar engine so the whole kernel
  uses ONE activation table set (exp/ln) - no table switches
- LN squares on the vector engine (tensor_mul) instead of ACT Square
- post-attention phase (out-proj -> LN1 -> FFN -> LN2 -> QKV of next layer)
  pipelined in T-halves so DVE LayerNorm work hides under PE matmuls
- weights for layer l+1 prefetched during attention(l)
- single 7-bank PSUM footprint in the pipelined phase (pf0-2/mm/st tags)
"""

import numpy as np

L, H, DK, DV, DM, DH = 6, 12, 64, 64, 768, 3072
B, T = 8, 1024
N_CORES = 8
KD = DM // 128   # 6
KH = DH // 128   # 24
KT = T // 128    # 8
NT = T // 512    # 2
TH = 512         # T-half size
SCALE = DM ** 0.5
HV = DV + 1      # per-head V width incl. ones column


def _pos_embed():
    pos = np.arange(T, dtype=np.float32)[:, None]
    i = np.arange(DM)[None, :]
    exp = ((i // 2) * 2).astype(np.float32) / DM
    ang = pos / np.power(np.float32(10000.0), exp, dtype=np.float32)
    return np.where(i % 2 == 0, np.sin(ang), np.cos(ang)).astype(np.float32)


def _build(nl=L, debug=False):
    import concourse.tile as tile
    from concourse import bacc, mybir
    from contextlib import ExitStack

    f32 = mybir.dt.float32
    bf16 = mybir.dt.bfloat16
    f8 = mybir.dt.float8e4
    AF = mybir.ActivationFunctionType
    ALU = mybir.AluOpType
    DR = mybir.MatmulPerfMode.DoubleRow
    QKS = 32.0           # fp8 scale for q/k (cancels via exp scale)
    VS = 16.0            # fp8 scale for v and the ones column
    PTS = float(np.log(8.0))  # exp bias: pt = 8*exp(s) (cancels in normalize)
    HVP = 784            # padded KT row width (HV*H=780 -> %16 for DoubleRow)

    nc = bacc.Bacc("TRN2", target_bir_lowering=False, num_devices=N_CORES)

    xt_d = nc.dram_tensor("xt", [DM, T], f32, kind="ExternalInput")
    wq_d = nc.dram_tensor("wq", [nl, DM, H * DK], bf16, kind="ExternalInput")
    wk_d = nc.dram_tensor("wk", [nl, DM, H * DK], bf16, kind="ExternalInput")
    wv_d = nc.dram_tensor("wv", [nl, DM, H * DV], bf16, kind="ExternalInput")
    pw_d = nc.dram_tensor("pw", [nl, H * DV, DM], bf16, kind="ExternalInput")
    w1_d = nc.dram_tensor("w1", [nl, DM, DH], bf16, kind="ExternalInput")
    w2_d = nc.dram_tensor("w2", [nl, DH, DM], bf16, kind="ExternalInput")
    pb_d = nc.dram_tensor("pb", [nl, DM], f32, kind="ExternalInput")
    b1_d = nc.dram_tensor("b1", [nl, DH], f32, kind="ExternalInput")
    b2_d = nc.dram_tensor("b2", [nl, DM], f32, kind="ExternalInput")
    l1g_d = nc.dram_tensor("l1g", [nl, DM], f32, kind="ExternalInput")
    l1b_d = nc.dram_tensor("l1b", [nl, DM], f32, kind="ExternalInput")
    l2g_d = nc.dram_tensor("l2g", [nl, DM], f32, kind="ExternalInput")
    l2b_d = nc.dram_tensor("l2b", [nl, DM], f32, kind="ExternalInput")
    yt_d = nc.dram_tensor("yt", [DM, T], f32, kind="ExternalOutput")
    dbg = {}
    if debug:
        f8_ = mybir.dt.float8e4
        for nm, shape, dt in (("qT", [DM, T], f8_), ("kT", [DM, T], f8_),
                              ("va", [128, KT * 784], f8_),
                              ("oT", [DM, T], bf16), ("r1", [DM, T], bf16),
                              ("xln1", [DM, T], bf16), ("r2", [DM, T], bf16)):
            dbg[nm] = nc.dram_tensor(f"dbg_{nm}", shape, dt, kind="ExternalOutput")

    def vec_ap(d, l):  # [nl, DM] dram row l -> [128, KD]
        return d[l].rearrange("(k p) -> p k", p=128)

    def hs(c):
        return slice(c * TH, (c + 1) * TH)

    with tile.TileContext(nc) as tc, ExitStack() as ctx:
        const = ctx.enter_context(tc.tile_pool(name="const", bufs=1))
        prm = ctx.enter_context(tc.tile_pool(name="prm", bufs=2))
        strm = ctx.enter_context(tc.tile_pool(name="strm", bufs=3))
        qkp = ctx.enter_context(tc.tile_pool(name="qkp", bufs=1))
        vap = ctx.enter_context(tc.tile_pool(name="vap", bufs=1))
        otp = ctx.enter_context(tc.tile_pool(name="otp", bufs=1))
        wbig = ctx.enter_context(tc.tile_pool(name="wbig", bufs=1))
        fwp = ctx.enter_context(tc.tile_pool(name="fwp", bufs=2))
        htp = ctx.enter_context(tc.tile_pool(name="htp", bufs=1))
        sqp = ctx.enter_context(tc.tile_pool(name="sqp", bufs=1))
        stat = ctx.enter_context(tc.tile_pool(name="stat", bufs=1))
        aux = ctx.enter_context(tc.tile_pool(name="aux", bufs=2))
        bcp = ctx.enter_context(tc.tile_pool(name="bcp", bufs=2))
        nrm = ctx.enter_context(tc.tile_pool(name="nrm", bufs=2))
        ppool = ctx.enter_context(tc.tile_pool(name="ppool", bufs=3))
        lnt = ctx.enter_context(tc.tile_pool(name="lnt", bufs=1))
        rfp = ctx.enter_context(tc.tile_pool(name="rfp", bufs=1))

        ones_b = const.tile([128, 1], bf16)
        nc.vector.memset(ones_b, 1.0)
        eps_sb = const.tile([1, 1], f32)
        nc.vector.memset(eps_sb, 1e-5)
        ln8_sb = const.tile([128, 1], f32)
        nc.vector.memset(ln8_sb, PTS)

        def load_layer_params(l):
            lp = prm.tile([128, 7 * KD], f32, tag="lp", name="lp")
            for i, d in enumerate((pb_d, b2_d, l1g_d, l1b_d, l2g_d, l2b_d)):
                nc.sync.dma_start(out=lp[:, i * KD:(i + 1) * KD], in_=vec_ap(d, l))
            nc.vector.tensor_scalar_mul(
                lp[:, 6 * KD:7 * KD], lp[:, 2 * KD:3 * KD], -1.0)
            b1_sb = prm.tile([128, KH + KD], f32, tag="b1", name="b1sb")
            nc.sync.dma_start(
                out=b1_sb[:, 0:KH], in_=b1_d[l].rearrange("(k p) -> p k", p=128))
            nc.vector.tensor_scalar_mul(
                b1_sb[:, KH:KH + KD], lp[:, 4 * KD:5 * KD], -1.0)
            return {
                "pb": lp[:, 0:KD], "b2": lp[:, KD:2 * KD],
                "g1": lp[:, 2 * KD:3 * KD], "bb1": lp[:, 3 * KD:4 * KD],
                "g2": lp[:, 4 * KD:5 * KD], "bb2": lp[:, 5 * KD:6 * KD],
                "gneg1": lp[:, 6 * KD:7 * KD], "gneg2": b1_sb[:, KH:KH + KD],
                "b1": b1_sb[:, 0:KH],
            }

        def load_qkvw(l, names=("wq", "wk", "wv", "pw")):
            dmap = {"wq": wq_d, "wk": wk_d, "wv": wv_d, "pw": pw_d}
            w = {}
            for nm in names:
                t = wbig.tile([128, KD, DM], bf16, tag=nm, name=f"{nm}{l}")
                nc.sync.dma_start(
                    out=t, in_=dmap[nm][l].rearrange("(k p) m -> p k m", p=128))
                w[nm] = t
            return w

        def qkv_half(c, xsrc, qT, kT, va, wq, wk, wv, psB):
            for w_sb, dst in ((wq, qT), (wk, kT)):
                for m in range(KD):
                    ps = psB.tile([128, TH], f32, tag="mm", bufs=2, name="psa")
                    for k in range(KD):
                        nc.tensor.matmul(
                            ps, w_sb[:, k, m * 128:(m + 1) * 128],
                            xsrc[:, k, hs(c)],
                            start=(k == 0), stop=(k == KD - 1))
                    nc.vector.tensor_scalar_mul(dst[:, m, hs(c)], ps, QKS)
            for tk in range(c * 4, c * 4 + 4):
                # v in token-major layout, interleaved into va; two psum chunks
                for n0, nw, h0, hn in ((0, 512, 0, 8), (512, 256, 8, 4)):
                    ps = psB.tile([128, TH], f32, tag="mm", bufs=2, name="psv")
                    for k in range(KD):
                        nc.tensor.matmul(
                            ps[:, 0:nw], xsrc[:, k, tk * 128:(tk + 1) * 128],
                            wv[:, k, n0:n0 + nw],
                            start=(k == 0), stop=(k == KD - 1))
                    out_ap = va[:, tk, 0:780].rearrange(
                        "p (h v) -> p h v", v=HV)[:, h0:h0 + hn, 0:64]
                    in_ap = ps[:, 0:nw].rearrange("p (h v) -> p h v", v=64)
                    nc.vector.tensor_scalar_mul(out_ap, in_ap, VS)

        def attention(qT, kT, va, oT, psS, psO):
            for h in range(H):
                d, off = divmod(h, 2)
                off *= 64
                po = psO.tile([65, T], f32, tag="po", name="po")
                pts = []

                def st_step(tk, d=d, off=off, pts=pts):
                    ps = psS.tile([128, T], f32, tag="pss", name="pss")
                    for n in range(NT):
                        nc.tensor.matmul(
                            ps[:, n * 512:(n + 1) * 512],
                            kT[off:off + 64, d, tk * 128:(tk + 1) * 128],
                            qT[off:off + 64, d, n * 512:(n + 1) * 512])
                    if tk % 2 == 0:
                        pts.append(ppool.tile([128, 2, T], f8, tag="pt",
                                              bufs=3, name="pt"))
                    # scores carry a QKS^2 factor; pt = 8*exp(s/SCALE)
                    nc.scalar.activation(
                        pts[tk // 2][:, tk % 2, :], ps, AF.Exp,
                        scale=1.0 / (QKS * QKS * SCALE), bias=ln8_sb[:])

                def pv_pair(j, h=h, po=po, pts=pts):
                    for n in range(NT):
                        nc.tensor.matmul(
                            po[:, n * 512:(n + 1) * 512],
                            va[:, 2 * j:2 * j + 2, h * HV:(h + 1) * HV],
                            pts[j][:, :, n * 512:(n + 1) * 512],
                            perf_mode=DR,
                            start=(j == 0), stop=(j == KT // 2 - 1))

                for tk in range(6):
                    st_step(tk)
                for j in range(KT // 2 - 1):
                    pv_pair(j)
                    if 2 * j + 6 < KT:
                        st_step(2 * j + 6)
                        st_step(2 * j + 7)
                pv_pair(KT // 2 - 1)
                # custom-DVE ops misread PSUM on HW - stage the denominator
                # row through SBUF before the approx reciprocal
                dn = nrm.tile([1, T], f32, tag="dn", bufs=1, name="dn")
                nc.vector.tensor_copy(dn, po[64:65, :])
                rp = nrm.tile([1, T], f32, tag="rp", bufs=1, name="rp")
                nc.vector.reciprocal_approx_fast(out=rp, in_=dn)
                rb = nrm.tile([64, T], f32, tag="rb", bufs=1, name="rb")
                nc.gpsimd.partition_broadcast(rb, rp)
                nc.vector.tensor_mul(oT[off:off + 64, d, :], po[0:64, :], rb)

        def outproj_half(c, oT, pw, pb_sb, rf, r_b, psB):
            # rf holds the f32 carrier (LN2 output of the previous layer);
            # update it in place: rf = (proj + pb) + rf.  r_b gets the bf16
            # copy the LN stats matmuls read.
            for m in range(KD):
                ps = psB.tile([128, TH], f32, tag="mm", bufs=2, name="psc")
                for k in range(KD):
                    nc.tensor.matmul(
                        ps, pw[:, k, m * 128:(m + 1) * 128], oT[:, k, hs(c)],
                        start=(k == 0), stop=(k == KD - 1))
                nc.vector.scalar_tensor_tensor(
                    out=rf[:, m, hs(c)], in0=ps, scalar=pb_sb[:, m:m + 1],
                    in1=rf[:, m, hs(c)], op0=ALU.add, op1=ALU.add)
                nc.vector.tensor_copy(r_b[:, m, hs(c)], rf[:, m, hs(c)])

        def ln_stats_half(c, r, psB):
            """returns (rs_bc, mu_bc) broadcast tiles for this half."""
            sq = sqp.tile([128, KD, TH], bf16, tag="sq", name="sq")
            nc.vector.tensor_mul(sq, r[:, :, hs(c)], r[:, :, hs(c)])
            s1p = psB.tile([1, TH], f32, tag="st", bufs=2, name="s1p")
            s2p = psB.tile([1, TH], f32, tag="st", bufs=2, name="s2p")
            for k in range(KD):
                nc.tensor.matmul(s1p, ones_b, r[:, k, hs(c)],
                                 start=(k == 0), stop=(k == KD - 1))
            for k in range(KD):
                nc.tensor.matmul(s2p, ones_b, sq[:, k, :],
                                 start=(k == 0), stop=(k == KD - 1))
            mean = stat.tile([1, TH], f32, tag="mean", name="mean")
            nc.vector.tensor_scalar_mul(mean, s1p, 1.0 / DM)
            ms = aux.tile([1, TH], f32, tag="aux", name="ms")
            nc.vector.tensor_scalar_mul(ms, s2p, 1.0 / DM)
            var = aux.tile([1, TH], f32, tag="aux", name="var")
            nc.vector.tensor_mul(var, mean, mean)
            nc.vector.tensor_sub(var, ms, var)
            # rstd = 1/sqrt(var+eps); Sqrt keeps the ACT table churn to one
            # set-switch per layer (vs Ln+Exp ping-pong), approx recip is cheap
            sd = aux.tile([1, TH], f32, tag="aux", name="sd")
            nc.scalar.activation(sd, var, AF.Sqrt, bias=eps_sb[:])
            rstd = aux.tile([1, TH], f32, tag="aux", name="rstd")
            nc.vector.reciprocal_approx_fast(out=rstd, in_=sd)
            murs = stat.tile([1, TH], f32, tag="murs", name="murs")
            nc.vector.tensor_mul(murs, mean, rstd)
            rs_bc = bcp.tile([128, TH], f32, tag="rs_bc", name="rs_bc")
            nc.gpsimd.partition_broadcast(rs_bc, rstd)
            mu_bc = bcp.tile([128, TH], f32, tag="mu_bc", name="mu_bc")
            nc.gpsimd.partition_broadcast(mu_bc, murs)
            return rs_bc, mu_bc

        def ln_apply_half(c, rf, rs_bc, mu_bc, g_sb, gneg_sb, b_sb, out_b,
                          yt_out=False):
            """rf[:,d,half] = ((rf*g)*rstd - g*mu*rstd) + b (in-place carrier
            update, f32); out_b gets the bf16 copy for matmul consumers.
            yt_out: stream the f32 carrier slice to yt dram (final layer)."""
            for d in range(KD):
                t = lnt.tile([128, TH], f32, tag="lt", name="lt")
                nc.vector.scalar_tensor_tensor(
                    out=t, in0=rf[:, d, hs(c)], scalar=g_sb[:, d:d + 1],
                    in1=rs_bc, op0=ALU.mult, op1=ALU.mult)
                u = lnt.tile([128, TH], f32, tag="lu", name="lu")
                nc.vector.scalar_tensor_tensor(
                    out=u, in0=mu_bc, scalar=gneg_sb[:, d:d + 1],
                    in1=t, op0=ALU.mult, op1=ALU.add)
                nc.vector.tensor_scalar(rf[:, d, hs(c)], u,
                                        b_sb[:, d:d + 1], None, ALU.add)
                if yt_out:
                    nc.sync.dma_start(
                        out=yt_d[:].rearrange(
                            "(k p) t -> p k t", p=128)[:, d, hs(c)],
                        in_=rf[:, d, hs(c)])
                else:
                    nc.vector.tensor_copy(out_b[:, d, hs(c)], rf[:, d, hs(c)])

        def ffn1_half(c, xln1, b1_sb, hT, l, psB):
            for j in range(12):  # 12 chunks of 2 m-columns each
                w1t = fwp.tile([128, KD, 256], bf16, tag="w1t", name="w1t")
                nc.sync.dma_start(
                    out=w1t,
                    in_=w1_d[l].rearrange(
                        "(k p) (a m) -> p k a m", p=128, m=256)[:, :, j, :])
                for mm in range(2):
                    m = j * 2 + mm
                    ps = psB.tile([128, TH], f32, tag="mm", bufs=2, name="pse")
                    for k in range(KD):
                        nc.tensor.matmul(
                            ps, w1t[:, k, mm * 128:(mm + 1) * 128],
                            xln1[:, k, hs(c)],
                            start=(k == 0), stop=(k == KD - 1))
                    nc.vector.tensor_scalar(
                        hT[:, m, :], ps, b1_sb[:, m:m + 1], 0.0,
                        ALU.add, ALU.max)

        def ffn2_half(c, hT, b2_sb, rf, r2b, l, psB):
            for g in range(2):  # two m-groups of 3 -> only 3 psum banks
                pf = [psB.tile([128, TH], f32, tag=f"pf{i}", name=f"pf{i}")
                      for i in range(3)]
                for kb in range(8):  # 8 chunks of 3 dh-rows each
                    w2t = fwp.tile([128, 3, 384], bf16, tag="w2t", name="w2t")
                    nc.sync.dma_start(
                        out=w2t,
                        in_=w2_d[l].rearrange(
                            "(b k p) m -> p b k m", k=3, p=128)
                        [:, kb, :, g * 384:(g + 1) * 384])
                    for k in range(3):
                        for i in range(3):
                            nc.tensor.matmul(
                                pf[i], w2t[:, k, i * 128:(i + 1) * 128],
                                hT[:, kb * 3 + k, :],
                                start=(kb == 0 and k == 0),
                                stop=(kb == 7 and k == 2))
                for i in range(3):
                    m = g * 3 + i
                    nc.vector.scalar_tensor_tensor(
                        out=rf[:, m, hs(c)], in0=pf[i],
                        scalar=b2_sb[:, m:m + 1],
                        in1=rf[:, m, hs(c)], op0=ALU.add, op1=ALU.add)
                    nc.vector.tensor_copy(r2b[:, m, hs(c)], rf[:, m, hs(c)])

        # ---- kernel start: load x, convert to bf16, first-layer weights ----
        w = load_qkvw(0)
        prms = load_layer_params(0)
        # rf is the persistent f32 residual carrier; starts as the input
        rf = rfp.tile([128, KD, T], f32, tag="rf", name="rf")
        nc.sync.dma_start(out=rf, in_=xt_d[:].rearrange("(k p) t -> p k t", p=128))
        xb = strm.tile([128, KD, T], bf16, tag="strm", name="xb0")
        nc.scalar.copy(xb, rf)

        qT = qkp.tile([128, KD, T], f8, tag="qT", name="qT")
        kT = qkp.tile([128, KD, T], f8, tag="kT", name="kT")
        va = vap.tile([128, KT, HVP], f8, tag="va", name="va")
        nc.vector.memset(
            va[:, :, 0:780].rearrange("p c (h v) -> p c h v", v=HV)[:, :, :, 64],
            VS)
        with tc.tile_pool(name="psB0", bufs=1, space="PSUM") as psB0:
            for c in range(2):
                qkv_half(c, xb, qT, kT, va, w["wq"], w["wk"], w["wv"], psB0)

        for l in range(nl):
            if debug and l == 0:
                nc.sync.dma_start(
                    out=dbg["qT"][:].rearrange("(k p) t -> p k t", p=128), in_=qT)
                nc.sync.dma_start(
                    out=dbg["kT"][:].rearrange("(k p) t -> p k t", p=128), in_=kT)
                nc.sync.dma_start(
                    out=dbg["va"][:].rearrange("p (c m) -> p c m", m=HVP),
                    in_=va)
            # ---- attention ----
            oT = otp.tile([128, KD, T], bf16, tag="oT", name="oT")
            pw_cur = w["pw"]
            with tc.tile_pool(name="psS", bufs=2, space="PSUM") as psS, \
                 tc.tile_pool(name="psO", bufs=2, space="PSUM") as psO:
                attention(qT, kT, va, oT, psS, psO)
            if debug and l == 0:
                nc.sync.dma_start(
                    out=dbg["oT"][:].rearrange("(k p) t -> p k t", p=128), in_=oT)
            # prefetch next layer weights + params (lands during attention/B).
            # pw(l+1) must wait: its buffer (bufs=1) is still read by this
            # layer's out-proj below - prefetch it after outproj_half(1).
            if l + 1 < nl:
                w = load_qkvw(l + 1, names=("wq", "wk", "wv"))
                next_prms = load_layer_params(l + 1)

            # ---- post-attention, pipelined by T-halves ----
            r1b = strm.tile([128, KD, T], bf16, tag="strm", name=f"r1_{l}")
            xln1 = strm.tile([128, KD, T], bf16, tag="strm", name=f"xln1_{l}")
            r2b = strm.tile([128, KD, T], bf16, tag="strm", name=f"r2_{l}")
            last = l + 1 == nl
            if not last:
                xn = strm.tile([128, KD, T], bf16, tag="strm", name=f"x_{l + 1}")
                qT = qkp.tile([128, KD, T], f8, tag="qT", name=f"qT{l + 1}")
                kT = qkp.tile([128, KD, T], f8, tag="kT", name=f"kT{l + 1}")
                va = vap.tile([128, KT, HVP], f8, tag="va", name=f"va{l + 1}")
                nc.vector.memset(
                    va[:, :, 0:780].rearrange(
                        "p c (h v) -> p c h v", v=HV)[:, :, :, 64], VS)
            else:
                xn = None

            with tc.tile_pool(name="psB", bufs=1, space="PSUM") as psB:
                outproj_half(0, oT, pw_cur, prms["pb"], rf, r1b, psB)
                st1_0 = ln_stats_half(0, r1b, psB)
                outproj_half(1, oT, pw_cur, prms["pb"], rf, r1b, psB)
                if l + 1 < nl:
                    w.update(load_qkvw(l + 1, names=("pw",)))
                st1_1 = ln_stats_half(1, r1b, psB)
                ln_apply_half(0, rf, *st1_0, prms["g1"], prms["gneg1"],
                              prms["bb1"], xln1)
                hT = htp.tile([128, KH, TH], bf16, tag="hT", name="hT0")
                ffn1_half(0, xln1, prms["b1"], hT, l, psB)
                ln_apply_half(1, rf, *st1_1, prms["g1"], prms["gneg1"],
                              prms["bb1"], xln1)
                ffn2_half(0, hT, prms["b2"], rf, r2b, l, psB)
                hT = htp.tile([128, KH, TH], bf16, tag="hT", name="hT1")
                ffn1_half(1, xln1, prms["b1"], hT, l, psB)
                st2_0 = ln_stats_half(0, r2b, psB)
                ffn2_half(1, hT, prms["b2"], rf, r2b, l, psB)
                st2_1 = ln_stats_half(1, r2b, psB)
                ln_apply_half(0, rf, *st2_0, prms["g2"], prms["gneg2"],
                              prms["bb2"], xn, yt_out=last)
                if not last:
                    qkv_half(0, xn, qT, kT, va, w["wq"], w["wk"], w["wv"], psB)
                ln_apply_half(1, rf, *st2_1, prms["g2"], prms["gneg2"],
                              prms["bb2"], xn, yt_out=last)
                if not last:
                    qkv_half(1, xn, qT, kT, va, w["wq"], w["wk"], w["wv"], psB)
                if debug and l == 0:
                    for nm, t in (("r1", r1b), ("xln1", xln1), ("r2", r2b)):
                        nc.sync.dma_start(
                            out=dbg[nm][:].rearrange("(k p) t -> p k t", p=128),
                            in_=t)
            if not last:
                prms = next_prms

    nc.compile()
    return nc


_NC = None


def _get_nc():
    global _NC
    if _NC is None:
        _NC = _build()
    return _NC


def _prep_inputs(inputs, nl=L):
    import ml_dtypes
    bf = ml_dtypes.bfloat16
    gi = lambda k: np.asarray(inputs[k])
    x = gi("x").astype(np.float32)
    wq, wk, wv = gi("wq"), gi("wk"), gi("wv")
    pe = _pos_embed()
    shared = {
        "wq": np.ascontiguousarray(wq[:nl].transpose(0, 2, 1, 3).reshape(nl, DM, H * DK)).astype(bf),
        "wk": np.ascontiguousarray(wk[:nl].transpose(0, 2, 1, 3).reshape(nl, DM, H * DK)).astype(bf),
        "wv": np.ascontiguousarray(wv[:nl].transpose(0, 2, 1, 3).reshape(nl, DM, H * DV)).astype(bf),
        "pw": np.ascontiguousarray(gi("proj_w")[:nl]).astype(bf),
        "w1": np.ascontiguousarray(gi("w1")[:nl]).astype(bf),
        "w2": np.ascontiguousarray(gi("w2")[:nl]).astype(bf),
        "pb": np.ascontiguousarray(gi("proj_b")[:nl], dtype=np.float32),
        "b1": np.ascontiguousarray(gi("b1")[:nl], dtype=np.float32),
        "b2": np.ascontiguousarray(gi("b2")[:nl], dtype=np.float32),
        "l1g": np.ascontiguousarray(gi("ln1_g")[:nl], dtype=np.float32),
        "l1b": np.ascontiguousarray(gi("ln1_b")[:nl], dtype=np.float32),
        "l2g": np.ascontiguousarray(gi("ln2_g")[:nl], dtype=np.float32),
        "l2b": np.ascontiguousarray(gi("ln2_b")[:nl], dtype=np.float32),
    }
    in_maps = []
    for b in range(B):
        m = dict(shared)
        m["xt"] = np.ascontiguousarray((x[b] + pe).T.astype(np.float32))
        in_maps.append(m)
    return in_maps


def run(inputs, trace=False):
    from concourse.bass_utils import run_bass_kernel_spmd
    nc = _get_nc()
    in_maps = _prep_inputs(inputs)
    res = run_bass_kernel_spmd(nc, in_maps, list(range(N_CORES)), trace=trace)
    out = np.stack([res.results[b]["yt"].T for b in range(B)]).astype(np.float32)
    return out, res


def kernel(**inputs):
    out, _ = run(inputs)
    return out
